# revision 7
# baseline (speedup 1.0000x reference)
"""Trainium2 Bass kernel for nn_CorrelationFilter (SiamFC-style correlation).

Math (per batch pair b):
    out[b, oi, oj] = sum_{di<6, dj<6, c<256} x[b, oi+di, oj+dj, c] * z[b, di, dj, c]
                     + sum_{c<256} bias[0, oi, oj, b*256 + c]
with x: [B,22,22,256], z: [B,6,6,256], bias: [1,17,17,B*256], out: [B,17,17,1].

Strategy: pure data parallelism over batch across 8 NeuronCores (16 batches per
core), no cross-core communication. Host does sharding + layout prep (transpose
to channel-major, cast to bf16) and pre-reduces the bias over its channel axis
(mathematically exact: bias enters the output only via sum_c).

Per-core layouts (DM = 3 di's merged per block, NK = 2 blocks, G = 18 groups;
group order g = djH*9 + dd*3 + dja with dj = 3*djH + dja):
  xT [2,128,16,484]      : xT[ch,c,b,p] = x[b, p//22, p%22, ch*128+c]
  zT [2,NK,128,16,G]     : zT[ch,k,c,b,g] = z[b, 3k+dd, dj, ch*128+c]
  bsum [16,289]          : bsum[b,o] = sum_c bias[0, o//17, o%17, b*256+c]

Device, one PSUM bank per batch:
  - 4 matmuls (ch,k): stationary zT[ch,k,:,b,:] (K=128, M=18), moving
    xT[ch][:, b, 66k : 66k+418], accumulating:
      Q[g, m] = group (djH,dd,dja) partial at column shift 22dd + 3djH + dja
  - ScalarE evacuation [18, 418] PSUM -> SBUF with f32->bf16 cast
  - two SBUF->SBUF DMAs put the djH=0 rows on partition b and the djH=1 rows
    (source-shifted by 3 cols, absorbing the constant djH shift) on partition
    16+b of t32 -> each fold add then handles TWO groups per op
  - fold: 8 pair-adds [32, 369] split Vector/GpSimd, cross-half merge, final
    add vs bsum view -> out[b,17,17] f32

kernel(**inputs) takes FULL unsharded inputs, returns the full output.
"""

import os
import numpy as np
import ml_dtypes

import concourse.bass as bass
import concourse.mybir as mybir
from concourse import bacc
from concourse.tile import TileContext

B, H, W, C = 128, 22, 22, 256
HZ, WZ = 6, 6
HO, WO = 17, 17
OO = HO * WO               # 289 dense output positions
NCORES = 8
BPC = B // NCORES          # 16 batches per core
P = H * W                  # 484 flattened search positions
O22 = (HO - 1) * W + WO    # 369: output span in 22-wide layout

DM = 3                     # di's merged per matmul block
NK = HZ // DM              # 2 matmul blocks per (ch)
G = DM * WZ                # 18 fold groups per batch
GH = G // 2                # 9 groups per dj-half
NMOV = O22 + (DM - 1) * W + (WZ - 1)  # 418 moving cols per matmul
HIW = NMOV - 3             # 415 cols kept of the dj-hi half

# fold split: vector takes the first NV pair-slots, gpsimd the rest
NV = 5

_BF16 = mybir.dt.bfloat16
_F32 = mybir.dt.float32


def build_module():
    nc = bacc.Bacc()
    xt_d = nc.dram_tensor("xt", [2, 128, BPC, P], _BF16, kind="ExternalInput")
    zt_d = nc.dram_tensor("zt", [2, NK, 128, BPC, G], _BF16, kind="ExternalInput")
    bs_d = nc.dram_tensor("bs", [BPC, OO], _BF16, kind="ExternalInput")
    out_d = nc.dram_tensor("out", [BPC, HO, WO], _F32, kind="ExternalOutput")

    with TileContext(nc) as tc:
        with (
            tc.tile_pool(name="const", bufs=1) as cpool,
            tc.tile_pool(name="big", bufs=1) as big,
            tc.tile_pool(name="evac", bufs=6) as epool,
            tc.tile_pool(name="work", bufs=1) as work,
            tc.tile_pool(name="psum", bufs=8, space="PSUM") as psum,
        ):
            # stationary z: [c, ch, k, b, g]
            zt_t = cpool.tile([128, 2, NK, BPC, G], _BF16, name="ztt")
            nc.scalar.dma_start(
                out=zt_t[:], in_=zt_d[:].rearrange("h k c b g -> c h k b g")
            )
            bsum = cpool.tile([BPC, OO], _BF16, name="bsum")
            nc.scalar.dma_start(out=bsum[:], in_=bs_d[:])

            xt_t = [
                big.tile([128, BPC, P], _BF16, name=f"xt{ch}", tag=f"xt{ch}")
                for ch in range(2)
            ]
            # t32[h*16 + b, j, m]: pair slot j = dd*3 + dja holds group
            # (djH=h, dd, dja), pre-aligned so both halves share shift
            # 22*dd + dja + 3 at read time
            t32 = big.tile([32, GH, NMOV], _BF16, name="t32")

            # xt chunk schedule: small first chunks to unblock matmul 0 early
            chunks = [(0, 2), (2, 2), (4, 4), (8, 4), (12, 4)]
            chunk_at = {c0: n for c0, n in chunks}

            for b in range(BPC):
                if b in chunk_at:
                    n = chunk_at[b]
                    for ch in range(2):
                        nc.sync.dma_start(
                            out=xt_t[ch][:, b : b + n, :],
                            in_=xt_d[ch, :, b : b + n, :],
                        )
                q1 = psum.tile([G, NMOV], _F32, name="q1", tag="q1", bufs=8)
                mms = [(ch, k) for ch in range(2) for k in range(NK)]
                for i, (ch, k) in enumerate(mms):
                    nc.tensor.matmul(
                        q1[:, :],
                        zt_t[:, ch, k, b, :],
                        xt_t[ch][:, b, DM * W * k : DM * W * k + NMOV],
                        start=(i == 0),
                        stop=(i == len(mms) - 1),
                    )
                eb = epool.tile([G, NMOV], _BF16, name="eb", tag="eb", bufs=6)
                nc.scalar.copy(out=eb[:], in_=q1[:])
                # dj-lo half -> partition b (full 418 cols)
                nc.gpsimd.dma_start(
                    out=t32[b : b + 1, :, :].rearrange("p g m -> p (g m)"),
                    in_=eb[0:GH, :],
                )
                # dj-hi half -> partition 16+b, source-shifted 3 cols
                nc.sync.dma_start(
                    out=t32[16 + b : 17 + b, :, 0:HIW],
                    in_=eb[GH:G, 3:NMOV],
                )

            # fold: 8 pair-adds, each covers groups (0,dd,dja) and (1,dd,dja)
            def j_src(j):
                dd, dja = j // 3, j % 3
                sh = 22 * dd + dja
                return t32[0:32, j, sh : sh + O22]

            accv = work.tile([32, O22 + WZ - 1], _BF16, name="accv")
            accg = work.tile([32, O22 + WZ - 1], _BF16, name="accg")
            av = accv[:, 0:O22]
            ag = accg[:, 0:O22]
            nc.vector.tensor_add(out=av, in0=j_src(0), in1=j_src(1))
            for j in range(2, NV):
                nc.vector.tensor_add(out=av, in0=av, in1=j_src(j))
            nc.gpsimd.tensor_add(out=ag, in0=j_src(NV), in1=j_src(NV + 1))
            for j in range(NV + 2, GH):
                nc.gpsimd.tensor_add(out=ag, in0=ag, in1=j_src(j))
            nc.vector.tensor_add(out=av, in0=av, in1=ag)
            # cross-half merge: engine operands can't start at partition 16,
            # so bounce the hi rows to a base-0 tile via SBUF->SBUF DMA
            accm = work.tile([BPC, O22], _BF16, name="accm")
            nc.sync.dma_start(out=accm[:], in_=accv[16:32, 0:O22])
            nc.vector.tensor_add(
                out=accv[0:BPC, 0:O22],
                in0=accv[0:BPC, 0:O22],
                in1=accm[:],
            )

            # final: dense 17x17 = acc (22-wide view) + bsum (dense view)
            outb = work.tile([BPC, HO, WO], _F32, name="outb")
            acc_v = accv[0:BPC, 0 : HO * W].rearrange("b (i j) -> b i j", j=W)[
                :, :, 0:WO
            ]
            bias_v = bsum[:].rearrange("b (i j) -> b i j", j=WO)
            nc.vector.tensor_add(out=outb[:], in0=acc_v, in1=bias_v)
            nc.sync.dma_start(out=out_d[:], in_=outb[:])

    nc.compile()
    return nc


def prep_inputs(x, z, b):
    """Host-side shard + layout prep. Returns per-core in_maps."""
    xb = np.asarray(x).astype(ml_dtypes.bfloat16)
    zb = np.asarray(z).astype(ml_dtypes.bfloat16)
    # exact: bias contributes to the output only through its channel sum
    bsum_all = (
        np.asarray(b).reshape(OO, B, C).sum(axis=2, dtype=np.float32)
    )  # [289, B]
    in_maps = []
    for core in range(NCORES):
        b0 = core * BPC
        xs = xb[b0 : b0 + BPC].reshape(BPC, P, C)
        xT = np.ascontiguousarray(xs.transpose(2, 0, 1)).reshape(2, 128, BPC, P)
        # zT[ch,k,c,b,g]: z[b, 3k+dd, 3*djH+dja, ch*128+c], g = djH*9+dd*3+dja
        zs = zb[b0 : b0 + BPC].reshape(BPC, NK, DM, 2, DM, C)  # b,k,dd,djH,dja,C
        zs = zs.transpose(5, 1, 0, 3, 2, 4)  # C,k,b,djH,dd,dja
        zT = np.ascontiguousarray(zs).reshape(2, 128, NK, BPC, G).transpose(
            0, 2, 1, 3, 4
        )
        zT = np.ascontiguousarray(zT)
        bs = np.ascontiguousarray(bsum_all[:, b0 : b0 + BPC].T).astype(
            ml_dtypes.bfloat16
        )
        in_maps.append({"xt": xT, "zt": zT, "bs": bs})
    return in_maps


_cache = {}


def _ensure_ntff_hook():
    """The axon NTFF profile hook normally lives in antenv.axon_hooks, which
    this image lacks; synthesize it from the boot shim's ctypes wrapper."""
    try:
        from antenv.axon_hooks import get_axon_ntff_profile_hook  # noqa: F401
        return True
    except ImportError:
        pass
    try:
        import sys, types
        from trn_agent_boot.trn_boot import _ntff_profile_via_ctypes

        so = os.environ.get("AXON_PJRT_SO", "/opt/axon/libaxon_pjrt.so")
        hook = _ntff_profile_via_ctypes(so)
        mod = types.ModuleType("antenv.axon_hooks")
        mod.get_axon_ntff_profile_hook = lambda: hook
        mod.set_axon_ntff_profile_hook = lambda h: None
        sys.modules["antenv.axon_hooks"] = mod
        import antenv

        antenv.axon_hooks = mod
        return True
    except Exception:
        return False


def kernel(x, z, b):
    from concourse.bass_utils import run_bass_kernel_spmd

    if "nc" not in _cache:
        _cache["nc"] = build_module()
    nc = _cache["nc"]
    in_maps = prep_inputs(x, z, b)
    trace = bool(int(os.environ.get("KERNEL_TRACE", "0") or 0))
    if trace:
        trace = _ensure_ntff_hook()
    res = run_bass_kernel_spmd(
        nc,
        in_maps,
        core_ids=list(range(NCORES)),
        trace=trace,
    )
    _cache["last_result"] = res
    out = np.concatenate([r["out"].reshape(BPC, HO, WO) for r in res.results], axis=0)
    return out[..., None].astype(np.float32)
